# revision 6
# baseline (speedup 1.0000x reference)
"""DETR scene-graph predicate head on 8 Trainium2 NeuronCores.

Math: logits[l,b,r,:] = concat(hs[l,b,q_sub], hs[l,b,q_obj]) @ W_pred.T + b_pred
with q_sub/q_obj derived from (tgt_perm inverse, relationships,
src_indices) — pure integer index math, done on host.

Strategy (batch axis sharded 8 ways; L*B/8 = 192 (layer,image) blocks/core),
project-first to keep the PE dense (HAM stays at 2.4 GHz) and DVE light:
  - hs ships transposed+unpadded as bf16 [128, 2, 101] per block (d-chunk on
    partitions); per-block PE matmuls project ALL queries against both W
    halves: proj[q,:] = [hs_q.Ws | hs_q.Wo] (2 accumulating N=102 matmuls,
    lhsT = hs^T chunk, rhs = Wcat), psum -> SBUF bf16 cast batched 4 blocks
    per bank on DVE/ACT.
  - Per-relation expansion on the PE: lg[r,p] = projS[q_sub[r],p] +
    projO[q_obj[r],51+p] via one-hot matmuls. The q_cat one-hot ships as
    fp8e4 bytes (1.0 = 0x38) and feeds lhsT directly (fp8 x bf16 matmul) —
    no cast, 1 byte/elem of DMA. Block pairs run col-packed via
    tile_position (0,0)/(0,64), outputs on psum partitions 0:64/64:128.
  - One bias add per group (DVE), bf16 stores batched 2 groups; host
    unpacks to [L, B, R, P] f32.

hs/W/proj bf16 (psum f32), one-hot fp8 (exact), logits stored bf16:
~6e-3 relative error vs the f32 reference.
"""

import sys

import numpy as np

L, B, Q1, D = 6, 256, 101, 256
M, R, P = 64, 64, 51
NCORES = 8
BLOC = B // NCORES          # images per core
NB = L * BLOC               # (layer, image) blocks per core
G = 8                       # blocks per group
NG = NB // G                # groups per core (24)
P2 = 2 * P                  # projected row width (102)
GP = G // 2                 # pairs per group (4)

_CACHE = {}


def _build_program():
    import concourse.bacc as bacc
    import concourse.mybir as mybir
    import concourse.tile as tile
    from contextlib import ExitStack

    f32 = mybir.dt.float32
    bf16 = mybir.dt.bfloat16
    f8 = mybir.dt.float8e4
    nc = bacc.Bacc("TRN2", target_bir_lowering=False, debug=False)

    hst = nc.dram_tensor("hst", [NG, 128, 2, G * Q1], bf16,
                         kind="ExternalInput").ap()
    ohq = nc.dram_tensor("ohq", [Q1, NG, G * 2 * R], f8,
                         kind="ExternalInput").ap()
    wcat = nc.dram_tensor("wcat", [128, 2, P2], bf16,
                          kind="ExternalInput").ap()
    bias = nc.dram_tensor("bias", [128, GP * P], f32,
                          kind="ExternalInput").ap()
    out = nc.dram_tensor("out", [128, NG, GP * P], bf16,
                         kind="ExternalOutput").ap()

    with tile.TileContext(nc) as tc, ExitStack() as ctx:
        const = ctx.enter_context(tc.tile_pool(name="const", bufs=1))
        hpool = ctx.enter_context(tc.tile_pool(name="hpool", bufs=3))
        opool = ctx.enter_context(tc.tile_pool(name="opool", bufs=2))
        prp = ctx.enter_context(tc.tile_pool(name="prp", bufs=4))
        outp = ctx.enter_context(tc.tile_pool(name="outp", bufs=2))
        psP = ctx.enter_context(tc.tile_pool(name="psP", bufs=4, space="PSUM"))
        psO = ctx.enter_context(tc.tile_pool(name="psO", bufs=3, space="PSUM"))

        wc_t = const.tile([128, 2, P2], bf16)
        nc.sync.dma_start(out=wc_t[:], in_=wcat[:])
        bias_t = const.tile([128, GP * P], f32)
        nc.sync.dma_start(out=bias_t[:], in_=bias[:])

        # HAM warm-up: ~3.4us of dense matmuls while the first loads stream
        wu = const.tile([128, 512], bf16)
        nc.vector.memset(wu[:], 0.0)
        for _ in range(12):
            wps = psP.tile([Q1, 4, P2], f32, tag="pP")
            nc.tensor.matmul(out=wps[0:Q1, 0, 0:P2], lhsT=wu[:, 0:Q1],
                             rhs=wu[:, 0:P2], start=True, stop=True)
            nc.tensor.matmul(out=wps[0:Q1, 2, 0:P2], lhsT=wu[:, 0:Q1],
                             rhs=wu[:, 0:P2], start=True, stop=True)

        for gg in range(NG // 2):
            oh_t = opool.tile([Q1, 2, G * 2 * R], f8, tag="oh")
            nc.sync.dma_start(out=oh_t[:], in_=ohq[:, 2 * gg:2 * gg + 2, :])
            o_t = outp.tile([128, 2, GP * P], bf16, tag="o")
            for g2 in range(2):
                g = 2 * gg + g2
                hst_t = hpool.tile([128, 2, G * Q1], bf16, tag="hst")
                nc.gpsimd.dma_start(out=hst_t[:], in_=hst[g])

                pO = psO.tile([128, GP * P], f32, tag="pO")
                for half in range(2):        # 4 blocks' proj share a bank
                    pP = psP.tile([Q1, 4, P2], f32, tag="pP")
                    for i in range(4):
                        j = half * 4 + i
                        for c in range(2):
                            nc.tensor.matmul(
                                out=pP[0:Q1, i, :],
                                lhsT=hst_t[:, c, j * Q1:(j + 1) * Q1],
                                rhs=wc_t[:, c, :],
                                start=(c == 0), stop=(c == 1))
                    pr = prp.tile([Q1, 4, P2], bf16, tag="pr")
                    if (g + half) % 2 == 0:
                        nc.vector.tensor_copy(out=pr[:], in_=pP[:])
                    else:
                        nc.scalar.copy(out=pr[:], in_=pP[:])

                    for k2 in range(2):
                        pk = half * 2 + k2
                        j0 = half * 4 + 2 * k2
                        i0, i1 = 2 * k2, 2 * k2 + 1
                        o0 = pO[0:R, pk * P:(pk + 1) * P]
                        o1 = pO[R:2 * R, pk * P:(pk + 1) * P]
                        for h in range(2):   # sub/obj halves accumulate
                            nc.tensor.matmul(
                                out=o0,
                                lhsT=oh_t[0:Q1, g2, j0 * 2 * R + h * R:
                                          j0 * 2 * R + (h + 1) * R],
                                rhs=pr[0:Q1, i0, h * P:(h + 1) * P],
                                start=(h == 0), stop=(h == 1),
                                tile_position=(0, 0))
                            nc.tensor.matmul(
                                out=o1,
                                lhsT=oh_t[0:Q1, g2, (j0 + 1) * 2 * R + h * R:
                                          (j0 + 1) * 2 * R + (h + 1) * R],
                                rhs=pr[0:Q1, i1, h * P:(h + 1) * P],
                                start=(h == 0), stop=(h == 1),
                                tile_position=(0, 64))

                nc.vector.tensor_add(out=o_t[:, g2, :], in0=pO[:],
                                     in1=bias_t[:])
            nc.scalar.dma_start(out=out[:, 2 * gg:2 * gg + 2, :], in_=o_t[:])

    nc.compile()
    return nc


def _host_indices(src_indices, tgt_perm, relationships):
    """q_sub, q_obj: [L, B, R] int64 — matched query slot per relation."""
    src = np.asarray(src_indices, dtype=np.int64)
    tgt = np.asarray(tgt_perm, dtype=np.int64)
    rel = np.asarray(relationships, dtype=np.int64)

    # lookup[l, b, tgt[l, b, k]] = k
    lookup = np.empty((L, B, M), dtype=np.int64)
    li = np.arange(L)[:, None, None]
    bi = np.arange(B)[None, :, None]
    lookup[li, bi, tgt] = np.broadcast_to(np.arange(M), (L, B, M))

    sub_t = np.broadcast_to(rel[None, :, :, 0], (L, B, R))
    obj_t = np.broadcast_to(rel[None, :, :, 1], (L, B, R))
    pos_sub = np.take_along_axis(lookup, sub_t, axis=2)
    pos_obj = np.take_along_axis(lookup, obj_t, axis=2)
    q_sub = np.take_along_axis(src, pos_sub, axis=2)
    q_obj = np.take_along_axis(src, pos_obj, axis=2)
    return q_sub, q_obj


def _host_prepare(hs, src_indices, tgt_perm, relationships, W_pred, b_pred):
    """Build per-core input maps."""
    import ml_dtypes
    bf16 = ml_dtypes.bfloat16
    f8 = ml_dtypes.float8_e4m3

    hs = np.asarray(hs, dtype=np.float32)
    W = np.asarray(W_pred, dtype=np.float32)
    b = np.asarray(b_pred, dtype=np.float32)

    q_sub, q_obj = _host_indices(src_indices, tgt_perm, relationships)
    q_cat = np.concatenate([q_sub, q_obj], axis=-1)           # [L, B, 2R]

    # Wcat[p, c, :] = [W_s^T | W_o^T] rows c*128+p
    WT = W.T                                                  # [2D, P]
    wcat = np.empty((2, 128, P2), dtype=np.float32)
    for c in range(2):
        wcat[c, :, 0:P] = WT[c * 128:(c + 1) * 128]
        wcat[c, :, P:P2] = WT[D + c * 128:D + (c + 1) * 128]
    wcat = np.ascontiguousarray(wcat.transpose(1, 0, 2)).astype(bf16)
    bias_b = np.ascontiguousarray(np.tile(b[None, :], (128, GP)))
    bias_b = bias_b.astype(np.float32)

    hs_bf = hs.astype(bf16)                                   # [L, B, Q1, D]

    in_maps = []
    for cc in range(NCORES):
        sl = slice(cc * BLOC, (cc + 1) * BLOC)
        hs_core = np.ascontiguousarray(hs_bf[:, sl]).reshape(NB, Q1, D)
        # hst[g, p, c, j*101+q] = hs[g*8+j, q, c*128+p]
        hstc = hs_core.reshape(NG, G, Q1, 2, 128).transpose(0, 4, 3, 1, 2)
        hstc = np.ascontiguousarray(hstc).reshape(NG, 128, 2, G * Q1)

        qc = np.ascontiguousarray(q_cat[:, sl]).reshape(NB, 2 * R)
        oh = (np.arange(Q1)[None, :, None] == qc[:, None, :])
        oh = (oh.astype(np.uint8) * 0x38)                     # fp8 1.0
        # ohq[q, g, j*128+r] = oh[g*8+j, q, r]
        ohc = oh.reshape(NG, G, Q1, 2 * R).transpose(2, 0, 1, 3)
        ohc = np.ascontiguousarray(ohc).reshape(Q1, NG, G * 2 * R)

        in_maps.append({
            "hst": hstc,
            "ohq": ohc.view(f8),
            "wcat": wcat,
            "bias": bias_b,
        })
    return in_maps


def kernel(hs, src_indices, tgt_perm, relationships, W_pred, b_pred):
    if "concourse" not in sys.modules:
        try:
            import concourse  # noqa: F401
        except ImportError:
            sys.path.insert(0, "/opt/trn_rl_repo")

    from concourse import bass_utils

    in_maps = _host_prepare(hs, src_indices, tgt_perm, relationships,
                            W_pred, b_pred)
    if "nc" not in _CACHE:
        _CACHE["nc"] = _build_program()
    nc = _CACHE["nc"]

    res = bass_utils.run_bass_kernel_spmd(nc, in_maps, list(range(NCORES)))
    outs = []
    for cc in range(NCORES):
        o = np.asarray(res.results[cc]["out"]).astype(np.float32)
        # out[p, g, pk*P + :P]; pair pk -> blocks g*8+2pk (parts 0:64) / +1
        o = o.reshape(2, R, NG, GP, P).transpose(2, 3, 0, 1, 4)
        outs.append(o.reshape(L, BLOC, R, P))
    return np.concatenate(outs, axis=1)
